# revision 1
# baseline (speedup 1.0000x reference)
"""ContrastiveCenterLoss Trainium2 Bass kernel.

Math
----
reference:  dis[b,c] = cos(hidden_b, center_c);  intra_b = dis[b, y_b];
            inter_b  = (sum_c dis[b,c] - intra_b) / (C-1)
            loss     = mean(1 - intra_b + inter_b)

Folded form used here (exact algebraic identities):
    cn_c    = fc_c / max(||fc_c||, eps)         (normalized centers)
    s       = sum_c cn_c
    invh_b  = 1 / max(||hidden_b||, eps)
    u_b     = hidden_b . cn_{y_b}               (gathered bf16 cn row)
    w       = sum_b invh_b * hidden_b           (PE matmul accumulation)
    loss    = 1 + [w.s - C * sum_b invh_b*u_b] / (B * (C-1))

The rowsum side collapses into one 128-d vector w (distributivity), so the
only per-sample work is two dot products (sumsq and u) plus a 256B-row
gather cn[y_b] — no [B,C] cosine matrix. The kernel is memory-bound: read
`hidden` once plus a bf16 gather.

Sharding: data-parallel over batch across 8 cores (4096 rows/core),
feature_center replicated; host sums the per-core partials:
    loss = 1 + (sum ws_c - C * sum partial_c) / (B*(C-1))

Engine budget per core: DVE runs the u dot products (fused mult+reduce TTR)
and half the q reductions; ACT does bulk squares; gpsimd does the gather
descriptor generation plus the other half of q via fused square+accum; PE
accumulates w. trn2 PE instructions allow only ONE sync wait, so PE operand
producers are kept on a single engine (DVE), with tiny self-referencing
"observer" matmuls to absorb the DMA-completion ticks first.
"""

import sys

sys.path.insert(0, "/opt/trn_rl_repo")

from contextlib import ExitStack

import numpy as np

import concourse.bass as bass
import concourse.tile as tile
from concourse import bacc, mybir
from concourse.bass import IndirectOffsetOnAxis
from concourse import library_config

B, C, D = 32768, 1000, 128
NCORES = 8
BS = B // NCORES          # 4096 rows per core
NT = BS // 128            # 32 batch tiles of 128 rows
CT = 8                    # center tiles
CP = C // CT              # 125 centers per tile
EPS = 1e-8
F32 = mybir.dt.float32
BF16 = mybir.dt.bfloat16
HCHUNKS = 8               # hidden-load / compute pipeline chunks
HTPC = NT // HCHUNKS
GCHUNKS = 2               # gather chunks
GTPC = NT // GCHUNKS
GPSIMD_Q_CHUNKS = 2       # earliest chunks' q on gpsimd, rest ACT+DVE
GPSIMD_U_TILES = 10       # trailing u tiles offloaded to gpsimd

_CACHED_NC = None


def build_nc() -> bass.Bass:
    AF = mybir.ActivationFunctionType
    OP = mybir.AluOpType

    nc = bacc.Bacc(dynamic_dma_scratch_size=65536)
    hidden = nc.dram_tensor("hidden", [BS, D], F32, kind="ExternalInput")
    fc = nc.dram_tensor("fc", [C, D], F32, kind="ExternalInput")
    yidx = nc.dram_tensor("yidx", [128, BS // 16], mybir.dt.int16, kind="ExternalInput")
    out_res = nc.dram_tensor("res", [128, 2], F32, kind="ExternalOutput")
    cn_dram = nc.dram_tensor("cn_table", [C, D], BF16)  # internal scratch

    with tile.TileContext(nc) as tc, ExitStack() as ctx:
        singles = ctx.enter_context(tc.tile_pool(name="singles", bufs=1))
        work = ctx.enter_context(tc.tile_pool(name="work", bufs=4))
        psum = ctx.enter_context(tc.tile_pool(name="psum", bufs=1, space="PSUM"))

        # Load the gpsimd library containing DMAGatherAnt (t=0, no deps).
        nc.gpsimd.load_library(library_config.mlp)

        # ---------------- phase 0: normalized-center table ----------------
        # fc rows (t*CP + p) -> fc_sb[p, t, :]
        fc_sb = singles.tile([CP, CT, D], F32)
        fc_src = fc[:, :].rearrange("(t p) d -> p t d", t=CT)
        nc.sync.dma_start(out=fc_sb[:, 0 : CT // 2, :], in_=fc_src[:, 0 : CT // 2, :])
        nc.scalar.dma_start(
            out=fc_sb[:, CT // 2 :, :], in_=fc_src[:, CT // 2 :, :]
        )
        # Preload the ACT sqrt/square function table early so the first real
        # activation doesn't pay the 1.3us table load on the critical chain.
        warm = singles.tile([128, 1], F32)
        nc.scalar.activation(out=warm, in_=warm, func=AF.Sqrt, scale=0.0, bias=1.0)

        # q_c[p, t] = ||fc row||^2, fused square+reduce per center tile (DVE)
        q_c = singles.tile([CP, CT], F32)
        for t in range(CT):
            prodc = work.tile([CP, D], F32, tag="prodc")
            nc.vector.scalar_tensor_tensor(
                out=prodc,
                in0=fc_sb[:, t, :],
                scalar=1.0,
                op0=OP.mult,
                in1=fc_sb[:, t, :],
                op1=OP.mult,
                accum_out=q_c[:, t : t + 1],
            )
        rt_c = singles.tile([CP, CT], F32)
        nc.scalar.activation(out=rt_c, in_=q_c, func=AF.Sqrt)
        nc.vector.tensor_scalar_max(out=rt_c, in0=rt_c, scalar1=EPS)
        inv_c = singles.tile([CP, CT], F32)
        nc.vector.reciprocal(out=inv_c, in_=rt_c)

        # cn = fc * inv_c, written directly as bf16 (table for gather + matmul)
        cn_bf = singles.tile([CP, CT, D], BF16)
        cn_dst = cn_dram[:, :].rearrange("(t p) d -> p t d", t=CT)
        for hh in range(2):
            t0, t1 = hh * (CT // 2), (hh + 1) * (CT // 2)
            nc.vector.tensor_tensor(
                out=cn_bf[:, t0:t1, :],
                in0=fc_sb[:, t0:t1, :],
                in1=inv_c[:, t0:t1].broadcast_to([CP, CT // 2, D]),
                op=OP.mult,
            )
            # store the table halves for the gather (ACT HWDGE queue)
            nc.scalar.dma_start(out=cn_dst[:, t0:t1, :], in_=cn_bf[:, t0:t1, :])

        # s[d] = sum_c cn[c, d]: ones-matmul partition reduction (off-path).
        ones_col = singles.tile([128, 1], BF16)
        nc.vector.memset(ones_col, 1.0)
        s_wide_ps = psum.tile([1, CT, D], F32)
        half = CT // 2  # keep each matmul's free size at 512 (one PSUM bank)
        for h in range(2):
            nc.tensor.matmul(
                out=s_wide_ps[:, h * half : (h + 1) * half, :],
                lhsT=ones_col[:CP, :],
                rhs=cn_bf[:, h * half : (h + 1) * half, :],
                start=True,
                stop=True,
            )
        s_sb = singles.tile([1, D], F32)
        nc.vector.tensor_reduce(
            out=s_sb[:, :],
            in_=s_wide_ps[:, :, :].rearrange("o t d -> o d t"),
            axis=mybir.AxisListType.X,
            op=OP.add,
        )

        # ---------------- main ----------------
        # h_all[p, i, :] = hidden[32*p + i, :]
        h_all = singles.tile([128, NT, D], F32)
        h_src = hidden[:, :].rearrange("(p i) d -> p i d", p=128)
        yi = singles.tile([128, BS // 16], mybir.dt.int16)
        nc.sync.dma_start(out=yi[:, :], in_=yidx[:, :])
        for k in range(HCHUNKS):
            j0, j1 = k * HTPC, (k + 1) * HTPC
            nc.sync.dma_start(out=h_all[:, j0:j1, :], in_=h_src[:, j0:j1, :])

        # gather cn[y] (bf16): SWDGE dma_gather on gpsimd (Tile-managed sems)
        cng = singles.tile([128, NT, D], BF16)
        NIC = BS // GCHUNKS  # indices per gather chunk
        for k in range(GCHUNKS):
            nc.gpsimd.dma_gather(
                out_ap=cng[:, k * GTPC : (k + 1) * GTPC, :],
                in_ap=cn_dram[:, :],
                idxs_ap=yi[:, k * (NIC // 16) : (k + 1) * (NIC // 16)],
                num_idxs=NIC,
                num_idxs_reg=NIC,
                elem_size=D,
                single_packet=False,
            )

        # PE "observer" matmuls: absorb each h-chunk's DMA tick so later
        # w-matmuls carry only a single (DVE) sync wait.
        junk_ps = psum.tile([1, 1], F32)
        for k in range(HCHUNKS):
            col = h_all[:, k * HTPC, 0:1]
            nc.tensor.matmul(
                out=junk_ps[:, :], lhsT=col, rhs=col, start=True, stop=True
            )

        q_all = singles.tile([128, NT], F32)
        z_all = singles.tile([128, NT], F32)
        inv_h = singles.tile([128, NT], F32)
        sq_late = singles.tile([128, NT, D], F32)
        w_ps = psum.tile([1, D], F32)

        # ---- phase A: q = ||h||^2 per chunk (bulk ACT square + DVE reduce) ----
        for k in range(HCHUNKS):
            j0, j1 = k * HTPC, (k + 1) * HTPC
            nc.scalar.activation(
                out=sq_late[:, j0:j1, :], in_=h_all[:, j0:j1, :], func=AF.Square
            )
            nc.vector.tensor_reduce(
                out=q_all[:, j0:j1],
                in_=sq_late[:, j0:j1, :],
                axis=mybir.AxisListType.X,
                op=OP.add,
            )

        # ---- phase B: invh = 1/max(sqrt(q), eps) per chunk ----
        for k in range(HCHUNKS):
            j0, j1 = k * HTPC, (k + 1) * HTPC
            nc.scalar.activation(
                out=inv_h[:, j0:j1], in_=q_all[:, j0:j1], func=AF.Sqrt
            )
            nc.vector.tensor_scalar_max(
                out=inv_h[:, j0:j1], in0=inv_h[:, j0:j1], scalar1=EPS
            )
            nc.vector.reciprocal(out=inv_h[:, j0:j1], in_=inv_h[:, j0:j1])

        # ---- phase C: u = h . cn[y] per tile (DVE fused mult+accum) ----
        for j in range(NT):
            prod = work.tile([128, D], F32, tag="prod")
            nc.vector.scalar_tensor_tensor(
                out=prod,
                in0=h_all[:, j, :],
                scalar=1.0,
                op0=OP.mult,
                in1=cng[:, j, :],
                op1=OP.mult,
                accum_out=z_all[:, j : j + 1],
            )

        # ---- phase D: w += invh_b * h_b (PE accumulation) ----
        for j in range(NT):
            nc.tensor.matmul(
                out=w_ps[:, :],
                lhsT=inv_h[:, j : j + 1],
                rhs=h_all[:, j, :],
                start=(j == 0),
                stop=(j == NT - 1),
                skip_group_check=True,
            )

        # ---------------- tail ----------------
        res_sb = singles.tile([128, 2], F32)
        nc.vector.memset(res_sb, 0.0)
        # ws = w . s  -> res[0, 1] (ready as soon as phase D finishes)
        wprod = singles.tile([1, D], F32)
        nc.vector.scalar_tensor_tensor(
            out=wprod,
            in0=w_ps[:, :],
            scalar=1.0,
            op0=OP.mult,
            in1=s_sb[:, :],
            op1=OP.mult,
            accum_out=res_sb[0:1, 1:2],
        )
        # partial[p] = sum_j z[p,j]*invh[p,j] -> res[:, 0] (one short op)
        vprod = singles.tile([128, NT], F32, tag="vprod")
        nc.vector.scalar_tensor_tensor(
            out=vprod,
            in0=z_all[:, :],
            scalar=1.0,
            op0=OP.mult,
            in1=inv_h[:, :],
            op1=OP.mult,
            accum_out=res_sb[:, 0:1],
        )
        nc.sync.dma_start(out=out_res[:, 0:2], in_=res_sb[:, 0:2])

    return nc


def _get_nc() -> bass.Bass:
    global _CACHED_NC
    if _CACHED_NC is None:
        _CACHED_NC = build_nc()
        _CACHED_NC.finalize()
    return _CACHED_NC


def _wrap_idx(y_shard: np.ndarray) -> np.ndarray:
    """dma_gather index layout: gather position i writes dst[i%128, i//128]
    and reads its index from idx[i%16, i//16] (replicated across the 8 Q7
    cores). We want dst slot (p, j) to hold sample 32*p + j so the gather
    output lines up with the hidden layout h_all[p, j, :] = hidden[32p+j]."""
    i = np.arange(BS)
    vals = y_shard[32 * (i % 128) + i // 128].astype(np.int16)
    idx16 = np.zeros((16, BS // 16), np.int16)
    idx16[i % 16, i // 16] = vals
    return np.ascontiguousarray(np.tile(idx16, (8, 1)))  # [128, BS//16]


def make_in_maps(hidden, feature_center, y):
    hidden = np.ascontiguousarray(np.asarray(hidden), dtype=np.float32)
    fc = np.ascontiguousarray(np.asarray(feature_center), dtype=np.float32)
    y32 = np.asarray(y).astype(np.int32)
    in_maps = []
    for c in range(NCORES):
        hs = hidden[c * BS : (c + 1) * BS]
        ys = _wrap_idx(y32[c * BS : (c + 1) * BS])
        in_maps.append({"hidden": hs, "fc": fc, "yidx": ys})
    return in_maps


def finish(results) -> np.ndarray:
    """results: list of dicts with 'res' [128,2]: col0 partials, [0,1] ws."""
    tot_u = 0.0
    tot_ws = 0.0
    for r in results:
        res = np.asarray(r["res"], dtype=np.float64)
        tot_u += res[:, 0].sum()
        tot_ws += res[0, 1]
    return np.float32(1.0 + (tot_ws - C * tot_u) / (B * (C - 1)))


def kernel(hidden, feature_center, y) -> np.ndarray:
    from concourse.bass_utils import run_bass_kernel_spmd

    in_maps = make_in_maps(hidden, feature_center, y)
    nc = _get_nc()
    res = run_bass_kernel_spmd(nc, in_maps, core_ids=list(range(NCORES)))
    return finish(res.results)



# revision 7
# speedup vs baseline: 1.8025x; 1.8025x over previous
"""ContrastiveCenterLoss Trainium2 Bass kernel (v2: sorted-window one-hot GEMM).

Math
----
reference:  dis[b,c] = cos(hidden_b, center_c);  intra_b = dis[b, y_b];
            inter_b  = (sum_c dis[b,c] - intra_b) / (C-1)
            loss     = mean(1 - intra_b + inter_b)

Folded form (exact algebraic identities):
    cn_c   = fc_c / max(||fc_c||, eps)
    s      = sum_c cn_c
    hn_b   = hidden_b / max(||hidden_b||, eps)
    G[c,:] = sum_{b: y_b=c} hn_b          (class-bucketed scatter sums)
    loss   = 1 + sum_{c,d} G^T[d,c] * (s[d] - C*cn[c,d]) / (B*(C-1))
because sum_c G[c] = sum_b hn_b recovers the rowsum term w.s and
sum_c G[c].cn_c is the intra term.

Device strategy (per core, data-parallel over batch):
    Host sorts the batch by class and shards contiguously, so each core's
    4096 samples span a narrow contiguous class window (~128 wide).  The
    scatter G^T = sum_tiles hn_tile^T @ onehot_tile then only needs a
    [128, W] one-hot per 128-row tile (W static, 160), built in one DVE
    is_equal op against an iota row.  G^T accumulates in PSUM across the
    32 tile matmuls; the loss contraction is a single fused
    multiply-accumulate against the host-precomputed fold table
    fold[d,w] = s[d] - C*cn[c0+w,d].  No gather, no gpsimd library.

    Engine budget: DVE builds both PE operands (hn bf16, one-hot bf16) so
    each matmul carries a single sync wait; gpsimd runs the q=||h||^2
    fused square-accumulate; ACT does rsqrt only.

Host does O(C*D) center normalization + O(B) sort/permute (layout only);
all O(B*D) per-sample math runs on device.  A W=1024 variant (no window
assumption, always correct) is built lazily if an input's sorted shards
ever span more than W classes.
"""

import sys

sys.path.insert(0, "/opt/trn_rl_repo")

from contextlib import ExitStack

import numpy as np

import concourse.bass as bass
import concourse.tile as tile
from concourse import bacc, mybir

B, C, D = 32768, 1000, 128
NCORES = 8
BS = B // NCORES          # 4096 rows per core
NT = BS // 128            # 32 batch tiles of 128 rows
W = 160                   # static class-window width per core
WFULL = 1024              # fallback width (covers any distribution)
EPS = 1e-8
INV_EPS = 1e8             # 1/EPS: min(rsqrt(q), 1/eps) == 1/max(sqrt(q), eps)
F32 = mybir.dt.float32
BF16 = mybir.dt.bfloat16
HCHUNKS = 8               # hidden-load pipeline chunks
HTPC = NT // HCHUNKS
DMA_QUEUES = None         # set in build_nc

_CACHED_NC = {}


def build_nc(width: int) -> bass.Bass:
    AF = mybir.ActivationFunctionType
    OP = mybir.AluOpType

    nc = bacc.Bacc()
    hidden = nc.dram_tensor("hidden", [BS, D], F32, kind="ExternalInput")
    yloc = nc.dram_tensor("yloc", [128, NT], F32, kind="ExternalInput")
    fold = nc.dram_tensor("fold", [128, width], F32, kind="ExternalInput")
    out_res = nc.dram_tensor("res", [128, 1], F32, kind="ExternalOutput")

    # matmul free-dim limit is 512; split wide windows into PSUM-bank halves
    nsplits = [(i, min(width, i + 512)) for i in range(0, width, 512)]

    with tile.TileContext(nc) as tc, ExitStack() as ctx:
        singles = ctx.enter_context(tc.tile_pool(name="singles", bufs=1))
        work = ctx.enter_context(tc.tile_pool(name="work", bufs=4))
        psum = ctx.enter_context(tc.tile_pool(name="psum", bufs=1, space="PSUM"))

        # ---------------- inputs ----------------
        yloc_sb = singles.tile([128, NT], F32)
        nc.scalar.dma_start(out=yloc_sb, in_=yloc[:, :])
        fold_sb = singles.tile([128, width], F32)
        nc.scalar.dma_start(out=fold_sb, in_=fold[:, :])

        # Preload the ACT sqrt table so the first invh doesn't pay the
        # 1.3us table load mid-pipeline.
        warm = singles.tile([128, 1], F32)
        nc.scalar.activation(out=warm, in_=warm, func=AF.Sqrt, scale=0.0, bias=1.0)

        # iota row 0..width-1, replicated on every partition (values are
        # small integers: exact in f32)
        iota_f = singles.tile([128, width], F32)
        nc.gpsimd.iota(
            iota_f,
            pattern=[[1, width]],
            base=0,
            channel_multiplier=0,
            allow_small_or_imprecise_dtypes=True,
        )

        # hidden tiles: h_all[p, i, :] = hidden[NT*p + i, :] — consecutive i
        # are contiguous rows, so each partition's chunk line is 2KB.
        # (Tile membership is irrelevant to the class window: the window
        # covers the whole sorted shard, so any sample-to-tile mapping works.)
        h_all = singles.tile([128, NT, D], F32)
        h_src = hidden[:, :].rearrange("(p i) d -> p i d", p=128)
        h_queues = [nc.sync, nc.scalar]
        for k in range(HCHUNKS):
            j0, j1 = k * HTPC, (k + 1) * HTPC
            h_queues[k % len(h_queues)].dma_start(
                out=h_all[:, j0:j1, :], in_=h_src[:, j0:j1, :]
            )

        # ---------------- per-tile compute ----------------
        q = singles.tile([128, NT], F32)
        invh = singles.tile([128, NT], F32)
        hn_bf = singles.tile([128, NT, D], BF16)
        ohs = singles.tile([128, NT, width], BF16)
        g_ps = psum.tile([128, width], F32)

        for k in range(HCHUNKS):
            j0, j1 = k * HTPC, (k + 1) * HTPC
            # q_j = ||h_j||^2 (ACT fused square+accum keeps DVE free)
            for j in range(j0, j1):
                sq = work.tile([128, D], BF16, tag="sq")
                nc.scalar.activation(
                    out=sq,
                    in_=h_all[:, j, :],
                    func=AF.Square,
                    accum_out=q[:, j : j + 1],
                )
            # invh = 1/max(sqrt(q), eps)
            nc.scalar.activation(
                out=invh[:, j0:j1], in_=q[:, j0:j1], func=AF.Sqrt
            )
            nc.vector.tensor_scalar_max(
                out=invh[:, j0:j1], in0=invh[:, j0:j1], scalar1=EPS
            )
            nc.vector.reciprocal(out=invh[:, j0:j1], in_=invh[:, j0:j1])
            for j in range(j0, j1):
                # hn = h * invh (per-partition scalar), cast to bf16
                nc.vector.tensor_scalar(
                    out=hn_bf[:, j, :],
                    in0=h_all[:, j, :],
                    scalar1=invh[:, j : j + 1],
                    scalar2=None,
                    op0=OP.mult,
                )
                # one-hot row: (iota == yloc_j), bf16 {0,1}
                nc.vector.tensor_scalar(
                    out=ohs[:, j, :],
                    in0=iota_f,
                    scalar1=yloc_sb[:, j : j + 1],
                    scalar2=None,
                    op0=OP.is_equal,
                )

        # ---------------- G^T accumulation on PE ----------------
        # G^T[d, w] += sum_p hn_bf[p, j, d] * ohs[p, j, w]
        for j in range(NT):
            for (w0, w1) in nsplits:
                nc.tensor.matmul(
                    out=g_ps[:, w0:w1],
                    lhsT=hn_bf[:, j, :],
                    rhs=ohs[:, j, w0:w1],
                    start=(j == 0),
                    stop=(j == NT - 1),
                    skip_group_check=True,
                )

        # ---------------- tail: N_partial[d] = sum_w G^T[d,w]*fold[d,w] ----
        res_sb = singles.tile([128, 1], F32)
        gprod = singles.tile([128, width], F32)
        nc.vector.scalar_tensor_tensor(
            out=gprod,
            in0=g_ps[:, :],
            scalar=1.0,
            op0=OP.mult,
            in1=fold_sb,
            op1=OP.mult,
            accum_out=res_sb[:, 0:1],
        )
        nc.sync.dma_start(out=out_res[:, :], in_=res_sb[:, :])

    return nc


def _get_nc(width: int) -> bass.Bass:
    if width not in _CACHED_NC:
        nc = build_nc(width)
        nc.finalize()
        _CACHED_NC[width] = nc
    return _CACHED_NC[width]


def _prep(hidden, feature_center, y):
    """Host-side layout + O(C*D) constant prep. Returns (in_maps, width)."""
    hidden = np.ascontiguousarray(np.asarray(hidden), dtype=np.float32)
    fc = np.asarray(feature_center, dtype=np.float32)
    y = np.asarray(y).astype(np.int64)

    cn = fc / np.maximum(np.linalg.norm(fc, axis=1, keepdims=True), EPS)
    s = cn.sum(0, dtype=np.float64).astype(np.float32)  # [D]

    perm = np.argsort(y, kind="stable")
    y_s = y[perm]
    h_s = hidden[perm]

    shard_width = max(
        int(y_s[(m + 1) * BS - 1] - y_s[m * BS] + 1) for m in range(NCORES)
    )
    width = W if shard_width <= W else WFULL

    in_maps = []
    for m in range(NCORES):
        ys = y_s[m * BS : (m + 1) * BS]
        hs = h_s[m * BS : (m + 1) * BS]
        c0 = 0 if width == WFULL else int(ys[0])
        yloc = (ys - c0).astype(np.float32).reshape(128, NT)  # yloc[p,i]=ys[NT*p+i]
        fold = np.tile(s[:, None], (1, width)).astype(np.float32)
        wreal = min(width, C - c0)
        fold[:, :wreal] = s[:, None] - C * cn[c0 : c0 + wreal].T
        in_maps.append(
            {
                "hidden": hs,
                "yloc": np.ascontiguousarray(yloc),
                "fold": np.ascontiguousarray(fold),
            }
        )
    return in_maps, width


def make_in_maps(hidden, feature_center, y):
    return _prep(hidden, feature_center, y)[0]


def finish(results) -> np.ndarray:
    """results: list of dicts with 'res' [128,1] per-feature partials."""
    tot = 0.0
    for r in results:
        tot += np.asarray(r["res"], dtype=np.float64).sum()
    return np.float32(1.0 + tot / (B * (C - 1)))


def kernel(hidden, feature_center, y) -> np.ndarray:
    from concourse.bass_utils import run_bass_kernel_spmd

    in_maps, width = _prep(hidden, feature_center, y)
    nc = _get_nc(width)
    res = run_bass_kernel_spmd(nc, in_maps, core_ids=list(range(NCORES)))
    return finish(res.results)


# revision 15
# speedup vs baseline: 2.2506x; 1.2486x over previous
"""ContrastiveCenterLoss Trainium2 Bass kernel (v3: sorted-window one-hot GEMM).

Math
----
reference:  dis[b,c] = cos(hidden_b, center_c);  intra_b = dis[b, y_b];
            inter_b  = (sum_c dis[b,c] - intra_b) / (C-1)
            loss     = mean(1 - intra_b + inter_b)

Folded form (exact algebraic identities):
    cn_c   = fc_c / max(||fc_c||, eps)
    s      = sum_c cn_c
    invh_b = 1 / max(||hidden_b||, eps)
    G[c,:] = sum_{b: y_b=c} invh_b * hidden_b     (class-bucketed scatter sums)
    loss   = 1 + sum_{c,d} G^T[d,c] * (s[d] - C*cn[c,d]) / (B*(C-1))
because sum_c G[c] = sum_b invh_b*hidden_b recovers the rowsum term w.s and
sum_c G[c].cn_c is the intra term.

Device strategy (per core, data-parallel over batch):
    Host sorts the batch by class and shards contiguously, so each core's
    4096 samples span a narrow contiguous class window (<=128 for near-
    uniform labels; W=144 static).  The scatter becomes a PE accumulation
    G^T += h_tile^T @ ohs_tile over 32 tiles of 128 samples, where
    ohs[b, w] = invh_b * (iota[w] == yloc_b) is built batched per 4-tile
    chunk in two DVE ops (tensor_tensor is_equal + broadcast multiply).
    The PE's stationary operand is the raw bf16 hidden tile straight from
    DMA (junk matmuls absorb the DMA-completion ticks so real matmuls
    carry a single DVE sync wait — trn2 PE allows only one).  The loss
    contraction fuses PSUM G^T against the host-precomputed fold table
    fold[d,w] = s[d] - C*cn[c0+w,d]; the [128,1] per-feature partials are
    collapsed to one scalar on the PE (a [128,1] column DMA would emit
    128 4-byte descriptors and stall the epilogue ~7us).

Host does O(C*D) center normalization + O(B) sort/permute + bf16 downcast
(layout/precision prep only); all O(B*D) per-sample math runs on device.
A W=1024 variant (no window assumption, always correct) is built lazily
if an input's sorted shards ever span more than W classes.
"""

import sys

sys.path.insert(0, "/opt/trn_rl_repo")

from contextlib import ExitStack

import ml_dtypes
import numpy as np

import concourse.bass as bass
import concourse.tile as tile
from concourse import bacc, mybir

B, C, D = 32768, 1000, 128
NCORES = 8
BS = B // NCORES          # 4096 rows per core
NT = BS // 128            # 32 batch tiles of 128 rows
W = 144                   # static class-window width per core
WFULL = 1024              # fallback width (covers any distribution)
EPS = 1e-8
F32 = mybir.dt.float32
BF16 = mybir.dt.bfloat16
HCHUNKS = 8               # hidden-load pipeline chunks
HTPC = NT // HCHUNKS      # 4 tiles per chunk

_CACHED_NC = {}


def build_nc(width: int) -> bass.Bass:
    AF = mybir.ActivationFunctionType
    OP = mybir.AluOpType

    nc = bacc.Bacc()
    # bf16 represents integers exactly only below 256; the WFULL fallback
    # compares class ids up to 1023 so it uses f32 iota/yloc.
    ydt = BF16 if width < 256 else F32
    hidden = nc.dram_tensor("hidden", [BS, D], BF16, kind="ExternalInput")
    yloc = nc.dram_tensor("yloc", [128, NT], ydt, kind="ExternalInput")
    fold = nc.dram_tensor("fold", [128, width], F32, kind="ExternalInput")
    out_res = nc.dram_tensor("res", [1, 1], F32, kind="ExternalOutput")

    # matmul free-dim limit is 512; split wide windows into PSUM-bank halves
    nsplits = [(i, min(width, i + 512)) for i in range(0, width, 512)]

    with tile.TileContext(nc) as tc, ExitStack() as ctx:
        singles = ctx.enter_context(tc.tile_pool(name="singles", bufs=1))
        work = ctx.enter_context(tc.tile_pool(name="work", bufs=4))
        psum = ctx.enter_context(tc.tile_pool(name="psum", bufs=1, space="PSUM"))

        # ---------------- DMA triggers first (queues stream async) -------
        # h_all[p, i, :] = hidden[NT*p + i, :] — consecutive i are contiguous
        # rows, so each partition's chunk line is 1KB (bf16).
        h_all = singles.tile([128, NT, D], BF16)
        h_src = hidden[:, :].rearrange("(p i) d -> p i d", p=128)
        yloc_sb = singles.tile([128, NT], ydt)
        fold_sb = singles.tile([128, width], F32)

        def h_chunk(k):
            j0, j1 = k * HTPC, (k + 1) * HTPC
            return dict(out=h_all[:, j0:j1, :], in_=h_src[:, j0:j1, :])

        # SP queue: h0, yloc, h2, h4, h6, fold; ACT queue: h1, h3, h5, h7.
        nc.sync.dma_start(**h_chunk(0))
        nc.scalar.dma_start(**h_chunk(1))
        nc.sync.dma_start(out=yloc_sb, in_=yloc[:, :])
        nc.scalar.dma_start(**h_chunk(3))
        nc.sync.dma_start(**h_chunk(2))
        nc.scalar.dma_start(**h_chunk(5))
        nc.sync.dma_start(**h_chunk(4))
        nc.scalar.dma_start(**h_chunk(7))
        nc.sync.dma_start(**h_chunk(6))
        nc.sync.dma_start(out=fold_sb, in_=fold[:, :])

        # iota ramp 0..width-1 (bf16-exact: width < 256), replicated over
        # every partition and over the HTPC tile axis
        iota_b = singles.tile([128, HTPC, width], ydt)
        nc.gpsimd.iota(
            iota_b,
            pattern=[[0, HTPC], [1, width]],
            base=0,
            channel_multiplier=0,
            allow_small_or_imprecise_dtypes=True,
        )
        ones_col = singles.tile([128, 1], F32)
        nc.vector.memset(ones_col, 1.0)

        # ---------------- per-chunk compute ----------------
        q = singles.tile([128, NT], F32)
        invh = singles.tile([128, NT], F32)
        ohs = singles.tile([128, NT, width], BF16)
        g_ps = psum.tile([128, width], F32)
        junk_ps = psum.tile([1, 1], F32)
        res_ps = psum.tile([1, 1], F32)

        for k in range(HCHUNKS):
            j0, j1 = k * HTPC, (k + 1) * HTPC
            # sq = h*h (ACT batched), q = rowsum(sq) (DVE reduce)
            sq = work.tile([128, HTPC, D], BF16, tag="sq")
            nc.scalar.activation(
                out=sq, in_=h_all[:, j0:j1, :], func=AF.Square
            )
            nc.vector.tensor_reduce(
                out=q[:, j0:j1],
                in_=sq,
                axis=mybir.AxisListType.X,
                op=OP.add,
            )
            # invh = 1/max(sqrt(q), eps)
            nc.scalar.activation(
                out=invh[:, j0:j1], in_=q[:, j0:j1], func=AF.Sqrt
            )
            nc.vector.tensor_scalar_max(
                out=invh[:, j0:j1], in0=invh[:, j0:j1], scalar1=EPS
            )
            nc.vector.reciprocal(out=invh[:, j0:j1], in_=invh[:, j0:j1])
            # oh = (iota == yloc), ohs = oh * invh  (both DVE, batched)
            oh = work.tile([128, HTPC, width], BF16, tag="oh")
            nc.vector.tensor_tensor(
                out=oh,
                in0=iota_b,
                in1=yloc_sb[:, j0:j1].broadcast_to([128, HTPC, width]),
                op=OP.is_equal,
            )
            nc.vector.scalar_tensor_tensor(
                out=ohs[:, j0:j1, :],
                in0=oh,
                scalar=1.0,
                op0=OP.mult,
                in1=invh[:, j0:j1].broadcast_to([128, HTPC, width]),
                op1=OP.mult,
            )

        # ---------------- G^T accumulation on PE ----------------
        # Junk matmuls absorb each chunk's DMA tick so real matmuls carry a
        # single (DVE) sync wait.  G^T[d, w] += sum_p h[p, j, d]*ohs[p, j, w]
        for k in range(HCHUNKS):
            j0, j1 = k * HTPC, (k + 1) * HTPC
            col = h_all[:, j0, 0:1]
            nc.tensor.matmul(
                out=junk_ps[:, :], lhsT=col, rhs=col,
                start=True, stop=True, skip_group_check=True,
            )
            for j in range(j0, j1):
                for (w0, w1) in nsplits:
                    nc.tensor.matmul(
                        out=g_ps[:, w0:w1],
                        lhsT=h_all[:, j, :],
                        rhs=ohs[:, j, w0:w1],
                        start=(j == 0),
                        stop=(j == NT - 1),
                        skip_group_check=True,
                    )

        # ---------------- tail ----------------
        # partial[d] = sum_w G^T[d,w]*fold[d,w]; then collapse the [128,1]
        # partials to one scalar on the PE (single-descriptor output DMA).
        part_sb = singles.tile([128, 1], F32)
        gprod = singles.tile([128, width], F32)
        nc.vector.scalar_tensor_tensor(
            out=gprod,
            in0=g_ps[:, :],
            scalar=1.0,
            op0=OP.mult,
            in1=fold_sb,
            op1=OP.mult,
            accum_out=part_sb[:, 0:1],
        )
        nc.tensor.matmul(
            out=res_ps[:, :], lhsT=ones_col, rhs=part_sb,
            start=True, stop=True, skip_group_check=True,
        )
        res_sb = singles.tile([1, 1], F32)
        nc.vector.tensor_copy(out=res_sb, in_=res_ps[:, :])
        nc.sync.dma_start(out=out_res[:, :], in_=res_sb[:, :])

    return nc


def _get_nc(width: int) -> bass.Bass:
    if width not in _CACHED_NC:
        nc = build_nc(width)
        nc.finalize()
        _CACHED_NC[width] = nc
    return _CACHED_NC[width]


def _prep(hidden, feature_center, y):
    """Host-side layout + O(C*D) constant prep. Returns (in_maps, width)."""
    hidden = np.ascontiguousarray(np.asarray(hidden), dtype=np.float32)
    fc = np.asarray(feature_center, dtype=np.float32)
    y = np.asarray(y).astype(np.int64)

    cn = fc / np.maximum(np.linalg.norm(fc, axis=1, keepdims=True), EPS)
    s = cn.sum(0, dtype=np.float64).astype(np.float32)  # [D]

    perm = np.argsort(y, kind="stable")
    y_s = y[perm]
    h_s = hidden[perm].astype(ml_dtypes.bfloat16)

    shard_width = max(
        int(y_s[(m + 1) * BS - 1] - y_s[m * BS] + 1) for m in range(NCORES)
    )
    width = W if shard_width <= W else WFULL

    in_maps = []
    for m in range(NCORES):
        ys = y_s[m * BS : (m + 1) * BS]
        hs = h_s[m * BS : (m + 1) * BS]
        c0 = 0 if width == WFULL else int(ys[0])
        # yloc[p,i] = ys[NT*p + i]; bf16 is integer-exact below 256, the
        # WFULL fallback (classes up to 1023) stays f32.
        yl = (ys - c0).astype(np.float32).reshape(128, NT)
        yloc = yl if width == WFULL else yl.astype(ml_dtypes.bfloat16)
        fold = np.tile(s[:, None], (1, width)).astype(np.float32)
        wreal = min(width, C - c0)
        fold[:, :wreal] = s[:, None] - C * cn[c0 : c0 + wreal].T
        in_maps.append(
            {
                "hidden": hs,
                "yloc": np.ascontiguousarray(yloc),
                "fold": np.ascontiguousarray(fold),
            }
        )
    return in_maps, width


def make_in_maps(hidden, feature_center, y):
    return _prep(hidden, feature_center, y)[0]


def finish(results) -> np.ndarray:
    """results: list of dicts with 'res' [1,1] per-core partial sums."""
    tot = 0.0
    for r in results:
        tot += float(np.asarray(r["res"], dtype=np.float64)[0, 0])
    return np.float32(1.0 + tot / (B * (C - 1)))


def kernel(hidden, feature_center, y) -> np.ndarray:
    from concourse.bass_utils import run_bass_kernel_spmd

    in_maps, width = _prep(hidden, feature_center, y)
    nc = _get_nc(width)
    res = run_bass_kernel_spmd(nc, in_maps, core_ids=list(range(NCORES)))
    return finish(res.results)


# revision 17
# speedup vs baseline: 2.3585x; 1.0479x over previous
"""ContrastiveCenterLoss Trainium2 Bass kernel (v3: sorted-window one-hot GEMM).

Math
----
reference:  dis[b,c] = cos(hidden_b, center_c);  intra_b = dis[b, y_b];
            inter_b  = (sum_c dis[b,c] - intra_b) / (C-1)
            loss     = mean(1 - intra_b + inter_b)

Folded form (exact algebraic identities):
    cn_c   = fc_c / max(||fc_c||, eps)
    s      = sum_c cn_c
    invh_b = 1 / max(||hidden_b||, eps)
    G[c,:] = sum_{b: y_b=c} invh_b * hidden_b     (class-bucketed scatter sums)
    loss   = 1 + sum_{c,d} G^T[d,c] * (s[d] - C*cn[c,d]) / (B*(C-1))
because sum_c G[c] = sum_b invh_b*hidden_b recovers the rowsum term w.s and
sum_c G[c].cn_c is the intra term.

Device strategy (per core, data-parallel over batch):
    Host sorts the batch by class and shards contiguously, so each core's
    4096 samples span a narrow contiguous class window (<=128 for near-
    uniform labels; W=144 static).  The scatter becomes a PE accumulation
    G^T += h_tile^T @ ohs_tile over 32 tiles of 128 samples, where
    ohs[b, w] = invh_b * (iota[w] == yloc_b) is built batched per 4-tile
    chunk in two DVE ops (tensor_tensor is_equal + broadcast multiply).
    The PE's stationary operand is the raw bf16 hidden tile straight from
    DMA (junk matmuls absorb the DMA-completion ticks so real matmuls
    carry a single DVE sync wait — trn2 PE allows only one).  The loss
    contraction fuses PSUM G^T against the host-precomputed fold table
    fold[d,w] = s[d] - C*cn[c0+w,d]; the [128,1] per-feature partials are
    collapsed to one scalar on the PE (a [128,1] column DMA would emit
    128 4-byte descriptors and stall the epilogue ~7us).

Host does O(C*D) center normalization + O(B) sort/permute + bf16 downcast
(layout/precision prep only); all O(B*D) per-sample math runs on device.
A W=1024 variant (no window assumption, always correct) is built lazily
if an input's sorted shards ever span more than W classes.
"""

import sys

sys.path.insert(0, "/opt/trn_rl_repo")

from contextlib import ExitStack

import ml_dtypes
import numpy as np

import concourse.bass as bass
import concourse.tile as tile
from concourse import bacc, mybir

B, C, D = 32768, 1000, 128
NCORES = 8
BS = B // NCORES          # 4096 rows per core
NT = BS // 128            # 32 batch tiles of 128 rows
W = 144                   # static class-window width per core
WFULL = 1024              # fallback width (covers any distribution)
EPS = 1e-8
F32 = mybir.dt.float32
BF16 = mybir.dt.bfloat16
HCHUNKS = 8               # hidden-load pipeline chunks
HTPC = NT // HCHUNKS      # 4 tiles per chunk

_CACHED_NC = {}


def build_nc(width: int) -> bass.Bass:
    AF = mybir.ActivationFunctionType
    OP = mybir.AluOpType

    nc = bacc.Bacc()
    # bf16 represents integers exactly only below 256; the WFULL fallback
    # compares class ids up to 1023 so it uses f32 iota/yloc.
    ydt = F32  # is_equal scalar operand must be f32
    hidden = nc.dram_tensor("hidden", [BS, D], BF16, kind="ExternalInput")
    yloc = nc.dram_tensor("yloc", [128, NT], ydt, kind="ExternalInput")
    fold = nc.dram_tensor("fold", [128, width], F32, kind="ExternalInput")
    out_res = nc.dram_tensor("res", [1, 1], F32, kind="ExternalOutput")

    # matmul free-dim limit is 512; split wide windows into PSUM-bank halves
    nsplits = [(i, min(width, i + 512)) for i in range(0, width, 512)]

    with tile.TileContext(nc) as tc, ExitStack() as ctx:
        singles = ctx.enter_context(tc.tile_pool(name="singles", bufs=1))
        work = ctx.enter_context(tc.tile_pool(name="work", bufs=4))
        psum = ctx.enter_context(tc.tile_pool(name="psum", bufs=1, space="PSUM"))

        # ---------------- DMA triggers first (queues stream async) -------
        # h_all[p, i, :] = hidden[NT*p + i, :] — consecutive i are contiguous
        # rows, so each partition's chunk line is 1KB (bf16).
        h_all = singles.tile([128, NT, D], BF16)
        h_src = hidden[:, :].rearrange("(p i) d -> p i d", p=128)
        yloc_sb = singles.tile([128, NT], ydt)
        fold_sb = singles.tile([128, width], F32)

        def h_chunk(k):
            j0, j1 = k * HTPC, (k + 1) * HTPC
            return dict(out=h_all[:, j0:j1, :], in_=h_src[:, j0:j1, :])

        # SP queue: h0, yloc, h2, h4, h6, fold; ACT queue: h1, h3, h5, h7.
        nc.sync.dma_start(**h_chunk(0))
        nc.scalar.dma_start(**h_chunk(1))
        nc.sync.dma_start(out=yloc_sb, in_=yloc[:, :])
        nc.scalar.dma_start(**h_chunk(3))
        nc.sync.dma_start(**h_chunk(2))
        nc.scalar.dma_start(**h_chunk(5))
        nc.sync.dma_start(**h_chunk(4))
        nc.scalar.dma_start(**h_chunk(7))
        nc.sync.dma_start(**h_chunk(6))
        nc.sync.dma_start(out=fold_sb, in_=fold[:, :])

        # iota ramp 0..width-1 (bf16-exact: width < 256), replicated over
        # every partition and over the HTPC tile axis
        iota_b = singles.tile([128, HTPC, width], ydt)
        nc.gpsimd.iota(
            iota_b,
            pattern=[[0, HTPC], [1, width]],
            base=0,
            channel_multiplier=0,
            allow_small_or_imprecise_dtypes=True,
        )
        ones_col = singles.tile([128, 1], F32)
        nc.vector.memset(ones_col, 1.0)

        # ---------------- per-chunk compute ----------------
        q = singles.tile([128, NT], F32)
        invh = singles.tile([128, NT], F32)
        ohs = singles.tile([128, NT, width], BF16)
        g_ps = psum.tile([128, width], F32)
        junk_ps = psum.tile([1, 1], F32)
        res_ps = psum.tile([1, 1], F32)

        for k in range(HCHUNKS):
            j0, j1 = k * HTPC, (k + 1) * HTPC
            # sq = h*h (ACT batched), q = rowsum(sq) (DVE reduce)
            sq = work.tile([128, HTPC, D], BF16, tag="sq")
            nc.scalar.activation(
                out=sq, in_=h_all[:, j0:j1, :], func=AF.Square
            )
            nc.vector.tensor_reduce(
                out=q[:, j0:j1],
                in_=sq,
                axis=mybir.AxisListType.X,
                op=OP.add,
            )
            # invh = 1/max(sqrt(q), eps)
            nc.scalar.activation(
                out=invh[:, j0:j1], in_=q[:, j0:j1], func=AF.Sqrt
            )
            nc.vector.tensor_scalar_max(
                out=invh[:, j0:j1], in0=invh[:, j0:j1], scalar1=EPS
            )
            nc.vector.reciprocal(out=invh[:, j0:j1], in_=invh[:, j0:j1])
            # ohs = (iota == yloc_j) * invh_j — one single-stream DVE op per
            # tile (two-stream tensor_tensor runs at half the DVE rate)
            for j in range(j0, j1):
                nc.vector.tensor_scalar(
                    out=ohs[:, j, :],
                    in0=iota_b[:, 0, :],
                    scalar1=yloc_sb[:, j : j + 1],
                    scalar2=invh[:, j : j + 1],
                    op0=OP.is_equal,
                    op1=OP.mult,
                )

        # ---------------- G^T accumulation on PE ----------------
        # Junk matmuls absorb each chunk's DMA tick so real matmuls carry a
        # single (DVE) sync wait.  G^T[d, w] += sum_p h[p, j, d]*ohs[p, j, w]
        for k in range(HCHUNKS):
            j0, j1 = k * HTPC, (k + 1) * HTPC
            col = h_all[:, j0, 0:1]
            nc.tensor.matmul(
                out=junk_ps[:, :], lhsT=col, rhs=col,
                start=True, stop=True, skip_group_check=True,
            )
            for j in range(j0, j1):
                for (w0, w1) in nsplits:
                    nc.tensor.matmul(
                        out=g_ps[:, w0:w1],
                        lhsT=h_all[:, j, :],
                        rhs=ohs[:, j, w0:w1],
                        start=(j == 0),
                        stop=(j == NT - 1),
                        skip_group_check=True,
                    )

        # ---------------- tail ----------------
        # partial[d] = sum_w G^T[d,w]*fold[d,w]; then collapse the [128,1]
        # partials to one scalar on the PE (single-descriptor output DMA).
        part_sb = singles.tile([128, 1], F32)
        gprod = singles.tile([128, width], F32)
        nc.vector.scalar_tensor_tensor(
            out=gprod,
            in0=g_ps[:, :],
            scalar=1.0,
            op0=OP.mult,
            in1=fold_sb,
            op1=OP.mult,
            accum_out=part_sb[:, 0:1],
        )
        nc.tensor.matmul(
            out=res_ps[:, :], lhsT=ones_col, rhs=part_sb,
            start=True, stop=True, skip_group_check=True,
        )
        res_sb = singles.tile([1, 1], F32)
        nc.vector.tensor_copy(out=res_sb, in_=res_ps[:, :])
        nc.sync.dma_start(out=out_res[:, :], in_=res_sb[:, :])

    return nc


def _get_nc(width: int) -> bass.Bass:
    if width not in _CACHED_NC:
        nc = build_nc(width)
        nc.finalize()
        _CACHED_NC[width] = nc
    return _CACHED_NC[width]


def _prep(hidden, feature_center, y):
    """Host-side layout + O(C*D) constant prep. Returns (in_maps, width)."""
    hidden = np.ascontiguousarray(np.asarray(hidden), dtype=np.float32)
    fc = np.asarray(feature_center, dtype=np.float32)
    y = np.asarray(y).astype(np.int64)

    cn = fc / np.maximum(np.linalg.norm(fc, axis=1, keepdims=True), EPS)
    s = cn.sum(0, dtype=np.float64).astype(np.float32)  # [D]

    perm = np.argsort(y, kind="stable")
    y_s = y[perm]
    h_s = hidden[perm].astype(ml_dtypes.bfloat16)

    shard_width = max(
        int(y_s[(m + 1) * BS - 1] - y_s[m * BS] + 1) for m in range(NCORES)
    )
    width = W if shard_width <= W else WFULL

    in_maps = []
    for m in range(NCORES):
        ys = y_s[m * BS : (m + 1) * BS]
        hs = h_s[m * BS : (m + 1) * BS]
        c0 = 0 if width == WFULL else int(ys[0])
        # yloc[p,i] = ys[NT*p + i]; bf16 is integer-exact below 256, the
        # WFULL fallback (classes up to 1023) stays f32.
        yl = (ys - c0).astype(np.float32).reshape(128, NT)
        yloc = yl
        fold = np.tile(s[:, None], (1, width)).astype(np.float32)
        wreal = min(width, C - c0)
        fold[:, :wreal] = s[:, None] - C * cn[c0 : c0 + wreal].T
        in_maps.append(
            {
                "hidden": hs,
                "yloc": np.ascontiguousarray(yloc),
                "fold": np.ascontiguousarray(fold),
            }
        )
    return in_maps, width


def make_in_maps(hidden, feature_center, y):
    return _prep(hidden, feature_center, y)[0]


def finish(results) -> np.ndarray:
    """results: list of dicts with 'res' [1,1] per-core partial sums."""
    tot = 0.0
    for r in results:
        tot += float(np.asarray(r["res"], dtype=np.float64)[0, 0])
    return np.float32(1.0 + tot / (B * (C - 1)))


def kernel(hidden, feature_center, y) -> np.ndarray:
    from concourse.bass_utils import run_bass_kernel_spmd

    in_maps, width = _prep(hidden, feature_center, y)
    nc = _get_nc(width)
    res = run_bass_kernel_spmd(nc, in_maps, core_ids=list(range(NCORES)))
    return finish(res.results)
